# revision 21
# baseline (speedup 1.0000x reference)
"""Trainium2 Bass kernel for rotated-filter-bank conv + channel sort + std.

Pipeline (per image): conv(x, 12 rotated 7x7 kernels, pad 3) -> leaky_relu
-> sort over the 12 channels per pixel -> concat unbiased std as channel 12.

Strategy: pure data parallel over 8 NeuronCores (2 images each).
Per core, per 8-row block: conv as one bf16 PE matmul with K = 14 input
rows x 7 taps = 98, M = 12 channels x 8 rows = 96, N = 512.  Two blocks
(one per super-block of a half-image) share one PSUM tile so the
leaky-relu eviction runs once per [96, 1024].  Channel-planar bf16 tiles
(gathered by SBUF->SBUF DMA, one DMA per channel per half-image) run a
41-comparator Batcher network on the vector engine (bf16 = 2x mode) with
a slice of comparators offloaded to the Pool engine.  Per-pixel sum(y)
and sum(y^2) accumulate on the tensor engine via identity matmuls (fp16
squares from the scalar engine) to form the unbiased std.  Outputs are
written in tile order as bf16 and unpermuted/widened on the host.
"""

import numpy as np
import ml_dtypes

KSIZE = 7
SIGMA = 3
CHANNELS = 12
H = W = 512
B = 16
N_CORES = 8
IMGS = B // N_CORES   # 2
R = 8                 # output rows per block
QROWS = R + 6         # input rows per block
KDIM = QROWS * 7      # 98
MDIM = CHANNELS * R   # 96
TB = 16               # blocks per super-block (128 rows)
NSB = 4               # super-blocks per image
HALVES = 2            # half-images processed per sort round
SBH = NSB // HALVES   # 2 super-blocks per half
FDW = SBH * W         # 1024 free elements in channel tiles
PADW = W + 6          # 518

# Batcher odd-even mergesort network for 12 elements (verified by 0-1 principle).
SORT_NET = [(0, 1), (2, 3), (0, 2), (1, 3), (1, 2), (4, 5), (6, 7), (4, 6),
            (5, 7), (5, 6), (0, 4), (2, 6), (2, 4), (1, 5), (3, 7), (3, 5),
            (1, 2), (3, 4), (5, 6), (8, 9), (10, 11), (8, 10), (9, 11),
            (9, 10), (0, 8), (4, 8), (2, 10), (6, 10), (2, 4), (6, 8),
            (1, 9), (5, 9), (3, 11), (7, 11), (3, 5), (7, 9), (1, 2),
            (3, 4), (5, 6), (7, 8), (9, 10)]

def _layered(net):
    """Group comparators into dependency layers for stall-free emission."""
    last = {}
    layers = []
    for ci, (i, j) in enumerate(net):
        lv = max(last.get(i, -1), last.get(j, -1)) + 1
        last[i] = last[j] = lv
        while len(layers) <= lv:
            layers.append([])
        layers[lv].append((ci, i, j))
    return layers


SORT_LAYERS = _layered(SORT_NET)

# Per layer, offload the last comparator to Pool when the layer is wide
# enough that DVE work covers Pool latency.
POOL_OFF = frozenset(
    layer[-1][0] for layer in SORT_LAYERS if len(layer) >= 4)


def _rotated_bank(kernel2d):
    """Replicates the reference affine_grid + grid_sample rotation in numpy."""
    lin = np.linspace(-1.0, 1.0, KSIZE)
    xs, ys = np.meshgrid(lin, lin)
    thetas = np.arange(CHANNELS) * np.pi / CHANNELS
    c = np.cos(thetas)[:, None, None]
    s = np.sin(thetas)[:, None, None]
    gx = (c * xs - s * ys).astype(np.float32)
    gy = (s * xs + c * ys).astype(np.float32)

    ix = (gx + np.float32(1.0)) * np.float32(0.5) * np.float32(KSIZE - 1)
    iy = (gy + np.float32(1.0)) * np.float32(0.5) * np.float32(KSIZE - 1)
    ix0 = np.floor(ix)
    iy0 = np.floor(iy)
    ix1 = ix0 + np.float32(1.0)
    iy1 = iy0 + np.float32(1.0)
    wx1 = ix - ix0
    wx0 = np.float32(1.0) - wx1
    wy1 = iy - iy0
    wy0 = np.float32(1.0) - wy1

    def gather(iyc, ixc):
        valid = ((ixc >= 0) & (ixc <= KSIZE - 1) & (iyc >= 0)
                 & (iyc <= KSIZE - 1)).astype(np.float32)
        iyi = np.clip(iyc, 0, KSIZE - 1).astype(np.int32)
        ixi = np.clip(ixc, 0, KSIZE - 1).astype(np.int32)
        return kernel2d[iyi, ixi] * valid

    rot = (gather(iy0, ix0) * wy0 * wx0 + gather(iy0, ix1) * wy0 * wx1 +
           gather(iy1, ix0) * wy1 * wx0 + gather(iy1, ix1) * wy1 * wx1)
    return rot.astype(np.float32)  # (12, 7, 7)


_RUNNER_CACHE = {}


def _build_runner():
    import bass_rust
    import concourse.tile as tile
    from concourse import bacc, mybir

    F32 = mybir.dt.float32
    F16 = mybir.dt.float16
    BF16 = mybir.dt.bfloat16
    Act = mybir.ActivationFunctionType
    Alu = mybir.AluOpType

    def V(pairs):
        return bass_rust.VecI64Pair(pairs)

    nc = bacc.Bacc("TRN2", target_bir_lowering=False, debug=False,
                   enable_asserts=False, num_devices=N_CORES)

    bx_d = nc.dram_tensor("bx", [IMGS, PADW, PADW], BF16, kind="ExternalInput")
    w0_d = nc.dram_tensor("w0", [KDIM, MDIM], BF16, kind="ExternalInput")
    idh_d = nc.dram_tensor("idh", [TB * R, TB * R], F16, kind="ExternalInput")
    # output in tile order: [img, ch, half, p=(r*16+t), f=(sbh*512+w)]
    y_d = nc.dram_tensor("y", [IMGS, CHANNELS + 1, HALVES, TB * R, FDW],
                         F16, kind="ExternalOutput")

    with tile.TileContext(nc) as tc:
        with tc.tile_pool(name="const", bufs=1) as cpool, \
             tc.tile_pool(name="rhs", bufs=4) as rpool, \
             tc.tile_pool(name="stage", bufs=2) as stpool, \
             tc.tile_pool(name="sq", bufs=2) as sqpool, \
             tc.tile_pool(name="ch", bufs=1) as chpool, \
             tc.tile_pool(name="aux", bufs=2) as xpool, \
             tc.tile_pool(name="pc", bufs=2, space="PSUM") as pcpool, \
             tc.tile_pool(name="sums", bufs=1, space="PSUM") as smpool:

            w0 = cpool.tile([KDIM, MDIM], BF16, tag="w0")
            nc.sync.dma_start(w0[:], w0_d.ap())
            idh = cpool.tile([TB * R, TB * R], F16, tag="idh")
            nc.sync.dma_start(idh[:], idh_d.ap())

            for img in range(IMGS):
                for half in range(HALVES):
                    # bf16 staging, block (t, sbh) at column (t*2+sbh)*512
                    stag = stpool.tile([MDIM, SBH * TB * W], F16, tag="stag")
                    stdt = xpool.tile([TB * R, FDW], F16, tag="std")

                    for t in range(TB):
                        pc = pcpool.tile([MDIM, SBH * W], F32, tag="pc")
                        for sbh in range(SBH):
                            sb = half * SBH + sbh
                            # conv rhs straight from HBM (im2col x7 taps):
                            # rhs[q*7+dx, w] = bx[img, sb*128+t*8+q, dx+w]
                            rhs = rpool.tile([KDIM, W], BF16,
                                             tag=f"rhs{sbh}", name="rhs")
                            src = bx_d.ap().copy()
                            src.offset = (img * PADW
                                          + (sb * TB + t) * R) * PADW
                            src.ap = V([[PADW, QROWS], [1, KSIZE], [1, W]])
                            nc.sync.dma_start(rhs[:], src)
                            nc.tensor.matmul(pc[:, sbh * W:(sbh + 1) * W],
                                             w0[:], rhs[:],
                                             start=True, stop=True)
                        # leaky-relu eviction of both blocks at once
                        nc.scalar.activation(
                            stag[:, t * SBH * W:(t + 1) * SBH * W], pc[:],
                            Act.Prelu, alpha=0.01)

                    # gather channel-planar bf16 tiles, one DMA per channel:
                    # ch[p=r*16+t, sbh*512+w] = stag[c*8+r, (t*2+sbh)*512+w]
                    chs = []
                    cur = [0] * CHANNELS

                    def fresh(c):
                        cur[c] ^= 1
                        return chpool.tile([TB * R, FDW], F16,
                                           name=f"srt{c}",
                                           tag=f"{'AB'[cur[c]]}{c}")

                    for c in range(CHANNELS):
                        cht = fresh(c)
                        gsrc = stag[:, :].copy()
                        gsrc.offset = (c * R) * (SBH * TB * W)
                        gsrc.ap = V([[SBH * TB * W, R], [W, SBH * TB],
                                     [1, W]])
                        nc.sync.dma_start(cht[:], gsrc)
                        chs.append(cht)

                    # per-pixel sums over channels via accumulating identity
                    # matmuls; fp16 squares on ACT feed sum(y^2).
                    S_ps = [smpool.tile([TB * R, W], F32, tag=f"S{i}",
                                        name=f"S{i}") for i in range(SBH)]
                    Y_ps = [smpool.tile([TB * R, W], F32, tag=f"Y{i}",
                                        name=f"Y{i}") for i in range(SBH)]
                    sqs = []
                    for c in range(CHANNELS):
                        sq = sqpool.tile([TB * R, FDW], F16, tag=f"sq{c % 3}",
                                         name="sq", bufs=3)
                        nc.scalar.activation(sq[:], chs[c][:], Act.Square)
                        sqs.append(sq)
                    for c in range(CHANNELS):
                        for i in range(SBH):
                            nc.tensor.matmul(S_ps[i][:], idh[:],
                                             chs[c][:, i * W:(i + 1) * W],
                                             start=(c == 0),
                                             stop=(c == CHANNELS - 1))
                    for c in range(CHANNELS):
                        for i in range(SBH):
                            nc.tensor.matmul(Y_ps[i][:], idh[:],
                                             sqs[c][:, i * W:(i + 1) * W],
                                             start=(c == 0),
                                             stop=(c == CHANNELS - 1))

                    # std = sqrt(relu(sum_y2 - S^2/12) / 11)
                    for i in range(SBH):
                        t1 = xpool.tile([TB * R, W], F32, tag="t1")
                        nc.scalar.activation(t1[:], S_ps[i][:], Act.Square)
                        v = xpool.tile([TB * R, W], F32, tag="v")
                        nc.vector.scalar_tensor_tensor(
                            v[:], t1[:], -1.0 / 12.0, Y_ps[i][:],
                            Alu.mult, Alu.add)
                        vc = xpool.tile([TB * R, W], F32, tag="vc")
                        nc.scalar.activation(vc[:], v[:], Act.Relu)
                        nc.scalar.activation(stdt[:, i * W:(i + 1) * W],
                                             vc[:], Act.Sqrt,
                                             scale=1.0 / 11.0)

                    # sorting network, layer by layer: within a layer all
                    # comparators are independent, so adjacent DVE ops never
                    # stall on each other.  Offloaded comparators compute
                    # max = (a+b) - min with the add/sub on Pool.
                    for layer in SORT_LAYERS:
                        news = {}
                        stiles = {}
                        for ci, i, j in layer:
                            if ci in POOL_OFF:
                                s = xpool.tile([TB * R, FDW], F16,
                                               tag=f"s{ci % 2}", name="s")
                                nc.gpsimd.tensor_tensor(s[:], chs[i][:],
                                                        chs[j][:], Alu.add)
                                stiles[ci] = s
                        for ci, i, j in layer:
                            mnt = fresh(i)
                            nc.vector.tensor_tensor(mnt[:], chs[i][:],
                                                    chs[j][:], Alu.min)
                            news[ci] = mnt
                        for ci, i, j in layer:
                            mxt = fresh(j)
                            if ci in POOL_OFF:
                                nc.gpsimd.tensor_tensor(
                                    mxt[:], stiles[ci][:], news[ci][:],
                                    Alu.subtract)
                            else:
                                nc.vector.tensor_tensor(mxt[:], chs[i][:],
                                                        chs[j][:], Alu.max)
                            chs[i] = news[ci]
                            chs[j] = mxt

                    # tile-order output DMAs (host unpermutes)
                    for k in range(CHANNELS + 1):
                        srctile = chs[k] if k < CHANNELS else stdt
                        od = y_d.ap().copy()
                        od.offset = ((img * (CHANNELS + 1) + k) * HALVES
                                     + half) * TB * R * FDW
                        od.ap = V([[FDW, TB * R], [1, FDW]])
                        nc.scalar.dma_start(od, srctile[:])

    nc.compile()
    return nc


def _get_nc():
    if "nc" not in _RUNNER_CACHE:
        _RUNNER_CACHE["nc"] = _build_runner()
    return _RUNNER_CACHE["nc"]


def _prep_inputs(x, kernel):
    """Host-side prep: rotate bank, build weights, pad + bf16 x."""
    rot = _rotated_bank(np.asarray(kernel, np.float32)[0, 0])

    # lhsT [98, 96]: W[q*7+dx, c*8+r] = rot[c, q-r, dx] for 0 <= q-r <= 6
    Wm = np.zeros((KDIM, MDIM), np.float32)
    for c in range(CHANNELS):
        for r in range(R):
            for dy in range(KSIZE):
                q = r + dy
                for dx in range(KSIZE):
                    Wm[q * KSIZE + dx, c * R + r] = rot[c, dy, dx]
    w0 = Wm.astype(ml_dtypes.bfloat16)
    idh = np.eye(TB * R, dtype=np.float16)

    x = np.asarray(x, np.float32)
    xp = np.zeros((B, PADW, PADW), np.float32)
    xp[:, SIGMA:SIGMA + H, SIGMA:SIGMA + W] = x[:, 0]
    xb = xp.astype(ml_dtypes.bfloat16)

    in_maps = []
    for core in range(N_CORES):
        i0 = core * IMGS
        in_maps.append({
            "bx": xb[i0:i0 + IMGS],
            "w0": w0,
            "idh": idh,
        })
    return in_maps


def run(in_maps, trace=False, **kwargs):
    from concourse import bass_utils
    nc = _get_nc()
    res = bass_utils.run_bass_kernel_spmd(
        nc, in_maps, core_ids=list(range(N_CORES)), trace=trace, **kwargs)
    return res


def _unpermute(y):
    # y: (IMGS, 13, HALVES, 128, 1024) with p = r*16 + t, f = sbh*512 + w
    y = y.reshape(IMGS, CHANNELS + 1, HALVES, R, TB, SBH, W)
    #                    img ch half  r  t  sbh  w -> img ch half sbh t r w
    y = y.transpose(0, 1, 2, 5, 4, 3, 6)
    return y.reshape(IMGS, CHANNELS + 1, H, W)


def kernel(x, kernel):
    in_maps = _prep_inputs(x, kernel)
    res = run(in_maps)
    y = np.stack([_unpermute(np.asarray(res.results[c]["y"]))
                  for c in range(N_CORES)])
    return y.reshape(B, CHANNELS + 1, H, W).astype(np.float32)


# revision 25
# speedup vs baseline: 1.1636x; 1.1636x over previous
"""Trainium2 Bass kernel for rotated-filter-bank conv + channel sort + std.

Pipeline (per image): conv(x, 12 rotated 7x7 kernels, pad 3) -> leaky_relu
-> sort over the 12 channels per pixel -> concat unbiased std as channel 12.

Strategy: pure data parallel over 8 NeuronCores (2 images each).
Per core, per 8-row block: conv as one bf16 PE matmul with K = 14 input
rows x 7 taps = 98, M = 12 channels x 8 rows = 96, N = 512.  Two blocks
(one per super-block of a half-image) share one PSUM tile so the
leaky-relu eviction runs once per [96, 1024].  Channel-planar bf16 tiles
(gathered by SBUF->SBUF DMA, one DMA per channel per half-image) run a
41-comparator Batcher network on the vector engine (bf16 = 2x mode) with
a slice of comparators offloaded to the Pool engine.  Per-pixel sum(y)
and sum(y^2) accumulate on the tensor engine via identity matmuls (fp16
squares from the scalar engine) to form the unbiased std.  Outputs are
written in tile order as bf16 and unpermuted/widened on the host.
"""

import numpy as np
import ml_dtypes

KSIZE = 7
SIGMA = 3
CHANNELS = 12
H = W = 512
B = 16
N_CORES = 8
IMGS = B // N_CORES   # 2
R = 8                 # output rows per block
QROWS = R + 6         # input rows per block
KDIM = QROWS * 7      # 98
MDIM = CHANNELS * R   # 96
TB = 16               # blocks per super-block (128 rows)
NSB = 4               # super-blocks per image
HALVES = 2            # half-images processed per sort round
SBH = NSB // HALVES   # 2 super-blocks per half
FDW = SBH * W         # 1024 free elements in channel tiles
PADW = W + 6          # 518

# Batcher odd-even mergesort network for 12 elements (verified by 0-1 principle).
SORT_NET = [(0, 1), (2, 3), (0, 2), (1, 3), (1, 2), (4, 5), (6, 7), (4, 6),
            (5, 7), (5, 6), (0, 4), (2, 6), (2, 4), (1, 5), (3, 7), (3, 5),
            (1, 2), (3, 4), (5, 6), (8, 9), (10, 11), (8, 10), (9, 11),
            (9, 10), (0, 8), (4, 8), (2, 10), (6, 10), (2, 4), (6, 8),
            (1, 9), (5, 9), (3, 11), (7, 11), (3, 5), (7, 9), (1, 2),
            (3, 4), (5, 6), (7, 8), (9, 10)]

def _layered(net):
    """Group comparators into dependency layers for stall-free emission."""
    last = {}
    layers = []
    for ci, (i, j) in enumerate(net):
        lv = max(last.get(i, -1), last.get(j, -1)) + 1
        last[i] = last[j] = lv
        while len(layers) <= lv:
            layers.append([])
        layers[lv].append((ci, i, j))
    return layers


SORT_LAYERS = _layered(SORT_NET)

# Pool TT ops are ~4x slower than DVE 2x-mode ops and consumers stall on
# them, so the sort stays entirely on DVE; Pool computes sum(y) instead.
POOL_OFF = frozenset()


def _rotated_bank(kernel2d):
    """Replicates the reference affine_grid + grid_sample rotation in numpy."""
    lin = np.linspace(-1.0, 1.0, KSIZE)
    xs, ys = np.meshgrid(lin, lin)
    thetas = np.arange(CHANNELS) * np.pi / CHANNELS
    c = np.cos(thetas)[:, None, None]
    s = np.sin(thetas)[:, None, None]
    gx = (c * xs - s * ys).astype(np.float32)
    gy = (s * xs + c * ys).astype(np.float32)

    ix = (gx + np.float32(1.0)) * np.float32(0.5) * np.float32(KSIZE - 1)
    iy = (gy + np.float32(1.0)) * np.float32(0.5) * np.float32(KSIZE - 1)
    ix0 = np.floor(ix)
    iy0 = np.floor(iy)
    ix1 = ix0 + np.float32(1.0)
    iy1 = iy0 + np.float32(1.0)
    wx1 = ix - ix0
    wx0 = np.float32(1.0) - wx1
    wy1 = iy - iy0
    wy0 = np.float32(1.0) - wy1

    def gather(iyc, ixc):
        valid = ((ixc >= 0) & (ixc <= KSIZE - 1) & (iyc >= 0)
                 & (iyc <= KSIZE - 1)).astype(np.float32)
        iyi = np.clip(iyc, 0, KSIZE - 1).astype(np.int32)
        ixi = np.clip(ixc, 0, KSIZE - 1).astype(np.int32)
        return kernel2d[iyi, ixi] * valid

    rot = (gather(iy0, ix0) * wy0 * wx0 + gather(iy0, ix1) * wy0 * wx1 +
           gather(iy1, ix0) * wy1 * wx0 + gather(iy1, ix1) * wy1 * wx1)
    return rot.astype(np.float32)  # (12, 7, 7)


_RUNNER_CACHE = {}


def _build_runner():
    import bass_rust
    import concourse.tile as tile
    from concourse import bacc, mybir

    F32 = mybir.dt.float32
    F16 = mybir.dt.float16
    BF16 = mybir.dt.bfloat16
    Act = mybir.ActivationFunctionType
    Alu = mybir.AluOpType

    def V(pairs):
        return bass_rust.VecI64Pair(pairs)

    nc = bacc.Bacc("TRN2", target_bir_lowering=False, debug=False,
                   enable_asserts=False, num_devices=N_CORES)

    bx_d = nc.dram_tensor("bx", [IMGS, PADW, PADW], BF16, kind="ExternalInput")
    w0_d = nc.dram_tensor("w0", [KDIM, MDIM], BF16, kind="ExternalInput")
    idh_d = nc.dram_tensor("idh", [TB * R, TB * R], F16, kind="ExternalInput")
    # output in tile order: [img, ch, half, p=(r*16+t), f=(sbh*512+w)]
    y_d = nc.dram_tensor("y", [IMGS, CHANNELS + 1, HALVES, TB * R, FDW],
                         F16, kind="ExternalOutput")

    with tile.TileContext(nc) as tc:
        with tc.tile_pool(name="const", bufs=1) as cpool, \
             tc.tile_pool(name="rhs", bufs=4) as rpool, \
             tc.tile_pool(name="stage", bufs=2) as stpool, \
             tc.tile_pool(name="sq", bufs=1) as sqpool, \
             tc.tile_pool(name="ch", bufs=1) as chpool, \
             tc.tile_pool(name="aux", bufs=1) as xpool, \
             tc.tile_pool(name="pc", bufs=2, space="PSUM") as pcpool, \
             tc.tile_pool(name="sums", bufs=1, space="PSUM") as smpool:

            w0 = cpool.tile([KDIM, MDIM], BF16, tag="w0")
            nc.sync.dma_start(w0[:], w0_d.ap())
            idh = cpool.tile([TB * R, TB * R], F16, tag="idh")
            nc.sync.dma_start(idh[:], idh_d.ap())

            for img in range(IMGS):
                for half in range(HALVES):
                    # bf16 staging, block (t, sbh) at column (t*2+sbh)*512
                    stag = stpool.tile([MDIM, SBH * TB * W], F16, tag="stag")
                    stdt = xpool.tile([TB * R, FDW], F16, tag="std", bufs=2)

                    for t in range(TB):
                        pc = pcpool.tile([MDIM, SBH * W], F32, tag="pc")
                        for sbh in range(SBH):
                            sb = half * SBH + sbh
                            # conv rhs straight from HBM (im2col x7 taps):
                            # rhs[q*7+dx, w] = bx[img, sb*128+t*8+q, dx+w]
                            rhs = rpool.tile([KDIM, W], BF16,
                                             tag=f"rhs{sbh}", name="rhs")
                            src = bx_d.ap().copy()
                            src.offset = (img * PADW
                                          + (sb * TB + t) * R) * PADW
                            src.ap = V([[PADW, QROWS], [1, KSIZE], [1, W]])
                            nc.sync.dma_start(rhs[:], src)
                            nc.tensor.matmul(pc[:, sbh * W:(sbh + 1) * W],
                                             w0[:], rhs[:],
                                             start=True, stop=True)
                        # leaky-relu eviction of both blocks at once
                        nc.scalar.activation(
                            stag[:, t * SBH * W:(t + 1) * SBH * W], pc[:],
                            Act.Prelu, alpha=0.01)

                    # gather channel-planar bf16 tiles, one DMA per channel:
                    # ch[p=r*16+t, sbh*512+w] = stag[c*8+r, (t*2+sbh)*512+w]
                    chs = []
                    cur = [0] * CHANNELS

                    def fresh(c):
                        cur[c] ^= 1
                        return chpool.tile([TB * R, FDW], F16,
                                           name=f"srt{c}",
                                           tag=f"{'AB'[cur[c]]}{c}")

                    for c in range(CHANNELS):
                        cht = fresh(c)
                        gsrc = stag[:, :].copy()
                        gsrc.offset = (c * R) * (SBH * TB * W)
                        gsrc.ap = V([[SBH * TB * W, R], [W, SBH * TB],
                                     [1, W]])
                        eng = nc.sync if c % 2 == 0 else nc.scalar
                        eng.dma_start(cht[:], gsrc)
                        chs.append(cht)

                    # sum(y) on the otherwise-idle Pool engine: fp16 pair
                    # sums (also layer-0 of the sort pairing) + fp32 tree.
                    ps = []
                    for p in range(6):
                        pt = sqpool.tile([TB * R, FDW], F16, tag=f"ps{p}",
                                         name="ps")
                        nc.gpsimd.tensor_tensor(pt[:], chs[2 * p][:],
                                                chs[2 * p + 1][:], Alu.add)
                        ps.append(pt)
                    q = []
                    for p in range(3):
                        qt = sqpool.tile([TB * R, FDW], F32, tag=f"qs{p}",
                                         name="qs")
                        nc.gpsimd.tensor_tensor(qt[:], ps[2 * p][:],
                                                ps[2 * p + 1][:], Alu.add)
                        q.append(qt)
                    q01 = sqpool.tile([TB * R, FDW], F32, tag="q01",
                                      name="q01")
                    nc.gpsimd.tensor_tensor(q01[:], q[0][:], q[1][:], Alu.add)
                    S_t = sqpool.tile([TB * R, FDW], F32, tag="S", name="S")
                    nc.gpsimd.tensor_tensor(S_t[:], q01[:], q[2][:], Alu.add)

                    # sum(y^2) via fp16 squares + accumulating identity
                    # matmuls on the tensor engine.
                    Y_ps = [smpool.tile([TB * R, W], F32, tag=f"Y{i}",
                                        name=f"Y{i}") for i in range(SBH)]
                    sqs = []
                    for c in range(CHANNELS):
                        sq = sqpool.tile([TB * R, FDW], F16, tag=f"sq{c % 3}",
                                         name="sq")
                        nc.scalar.activation(sq[:], chs[c][:], Act.Square)
                        sqs.append(sq)
                    for c in range(CHANNELS):
                        for i in range(SBH):
                            nc.tensor.matmul(Y_ps[i][:], idh[:],
                                             sqs[c][:, i * W:(i + 1) * W],
                                             start=(c == 0),
                                             stop=(c == CHANNELS - 1))

                    # std = sqrt(relu(sum_y2 - S^2/12) / 11)
                    for i in range(SBH):
                        t1 = xpool.tile([TB * R, W], F32, tag="t1")
                        nc.scalar.activation(t1[:],
                                             S_t[:, i * W:(i + 1) * W],
                                             Act.Square)
                        v = xpool.tile([TB * R, W], F32, tag="v")
                        nc.vector.scalar_tensor_tensor(
                            v[:], t1[:], -1.0 / 12.0, Y_ps[i][:],
                            Alu.mult, Alu.add)
                        vc = xpool.tile([TB * R, W], F32, tag="vc")
                        nc.scalar.activation(vc[:], v[:], Act.Relu)
                        nc.scalar.activation(stdt[:, i * W:(i + 1) * W],
                                             vc[:], Act.Sqrt,
                                             scale=1.0 / 11.0)

                    # sorting network, layer by layer: within a layer all
                    # comparators are independent, so adjacent DVE ops never
                    # stall on each other.  Offloaded comparators compute
                    # max = (a+b) - min with the add/sub on Pool.
                    for layer in SORT_LAYERS:
                        news = {}
                        stiles = {}
                        for ci, i, j in layer:
                            if ci in POOL_OFF:
                                s = xpool.tile([TB * R, FDW], F16,
                                               tag=f"s{ci % 2}", name="s")
                                nc.gpsimd.tensor_tensor(s[:], chs[i][:],
                                                        chs[j][:], Alu.add)
                                stiles[ci] = s
                        for ci, i, j in layer:
                            mnt = fresh(i)
                            nc.vector.tensor_tensor(mnt[:], chs[i][:],
                                                    chs[j][:], Alu.min)
                            news[ci] = mnt
                        for ci, i, j in layer:
                            mxt = fresh(j)
                            if ci in POOL_OFF:
                                nc.gpsimd.tensor_tensor(
                                    mxt[:], stiles[ci][:], news[ci][:],
                                    Alu.subtract)
                            else:
                                nc.vector.tensor_tensor(mxt[:], chs[i][:],
                                                        chs[j][:], Alu.max)
                            chs[i] = news[ci]
                            chs[j] = mxt

                    # tile-order output DMAs (host unpermutes)
                    for k in range(CHANNELS + 1):
                        srctile = chs[k] if k < CHANNELS else stdt
                        od = y_d.ap().copy()
                        od.offset = ((img * (CHANNELS + 1) + k) * HALVES
                                     + half) * TB * R * FDW
                        od.ap = V([[FDW, TB * R], [1, FDW]])
                        nc.scalar.dma_start(od, srctile[:])

    nc.compile()
    return nc


def _get_nc():
    if "nc" not in _RUNNER_CACHE:
        _RUNNER_CACHE["nc"] = _build_runner()
    return _RUNNER_CACHE["nc"]


def _prep_inputs(x, kernel):
    """Host-side prep: rotate bank, build weights, pad + bf16 x."""
    rot = _rotated_bank(np.asarray(kernel, np.float32)[0, 0])

    # lhsT [98, 96]: W[q*7+dx, c*8+r] = rot[c, q-r, dx] for 0 <= q-r <= 6
    Wm = np.zeros((KDIM, MDIM), np.float32)
    for c in range(CHANNELS):
        for r in range(R):
            for dy in range(KSIZE):
                q = r + dy
                for dx in range(KSIZE):
                    Wm[q * KSIZE + dx, c * R + r] = rot[c, dy, dx]
    w0 = Wm.astype(ml_dtypes.bfloat16)
    idh = np.eye(TB * R, dtype=np.float16)

    x = np.asarray(x, np.float32)
    xp = np.zeros((B, PADW, PADW), np.float32)
    xp[:, SIGMA:SIGMA + H, SIGMA:SIGMA + W] = x[:, 0]
    xb = xp.astype(ml_dtypes.bfloat16)

    in_maps = []
    for core in range(N_CORES):
        i0 = core * IMGS
        in_maps.append({
            "bx": xb[i0:i0 + IMGS],
            "w0": w0,
            "idh": idh,
        })
    return in_maps


def run(in_maps, trace=False, **kwargs):
    from concourse import bass_utils
    nc = _get_nc()
    res = bass_utils.run_bass_kernel_spmd(
        nc, in_maps, core_ids=list(range(N_CORES)), trace=trace, **kwargs)
    return res


def _unpermute(y):
    # y: (IMGS, 13, HALVES, 128, 1024) with p = r*16 + t, f = sbh*512 + w
    y = y.reshape(IMGS, CHANNELS + 1, HALVES, R, TB, SBH, W)
    #                    img ch half  r  t  sbh  w -> img ch half sbh t r w
    y = y.transpose(0, 1, 2, 5, 4, 3, 6)
    return y.reshape(IMGS, CHANNELS + 1, H, W)


def kernel(x, kernel):
    in_maps = _prep_inputs(x, kernel)
    res = run(in_maps)
    y = np.stack([_unpermute(np.asarray(res.results[c]["y"]))
                  for c in range(N_CORES)])
    return y.reshape(B, CHANNELS + 1, H, W).astype(np.float32)


# revision 27
# speedup vs baseline: 1.2720x; 1.0932x over previous
"""Trainium2 Bass kernel for rotated-filter-bank conv + channel sort + std.

Pipeline (per image): conv(x, 12 rotated 7x7 kernels, pad 3) -> leaky_relu
-> sort over the 12 channels per pixel -> concat unbiased std as channel 12.

Strategy: pure data parallel over 8 NeuronCores (2 images each).
Per core, per 8-row block: conv as one bf16 PE matmul with K = 14 input
rows x 7 taps = 98, M = 12 channels x 8 rows = 96, N = 512.  Two blocks
(one per super-block of a half-image) share one PSUM tile so the
leaky-relu eviction runs once per [96, 1024].  Channel-planar bf16 tiles
(gathered by SBUF->SBUF DMA, one DMA per channel per half-image) run a
41-comparator Batcher network on the vector engine (bf16 = 2x mode) with
a slice of comparators offloaded to the Pool engine.  Per-pixel sum(y)
and sum(y^2) accumulate on the tensor engine via identity matmuls (fp16
squares from the scalar engine) to form the unbiased std.  Outputs are
written in tile order as bf16 and unpermuted/widened on the host.
"""

import numpy as np
import ml_dtypes

KSIZE = 7
SIGMA = 3
CHANNELS = 12
H = W = 512
B = 16
N_CORES = 8
IMGS = B // N_CORES   # 2
R = 8                 # output rows per block
QROWS = R + 6         # input rows per block
KDIM = QROWS * 7      # 98
MDIM = CHANNELS * R   # 96
TB = 16               # blocks per super-block (128 rows)
NSB = 4               # super-blocks per image
HALVES = 2            # half-images processed per sort round
SBH = NSB // HALVES   # 2 super-blocks per half
FDW = SBH * W         # 1024 free elements in channel tiles
PADW = W + 6          # 518

# Batcher odd-even mergesort network for 12 elements (verified by 0-1 principle).
SORT_NET = [(0, 1), (2, 3), (0, 2), (1, 3), (1, 2), (4, 5), (6, 7), (4, 6),
            (5, 7), (5, 6), (0, 4), (2, 6), (2, 4), (1, 5), (3, 7), (3, 5),
            (1, 2), (3, 4), (5, 6), (8, 9), (10, 11), (8, 10), (9, 11),
            (9, 10), (0, 8), (4, 8), (2, 10), (6, 10), (2, 4), (6, 8),
            (1, 9), (5, 9), (3, 11), (7, 11), (3, 5), (7, 9), (1, 2),
            (3, 4), (5, 6), (7, 8), (9, 10)]

def _layered(net):
    """Group comparators into dependency layers for stall-free emission."""
    last = {}
    layers = []
    for ci, (i, j) in enumerate(net):
        lv = max(last.get(i, -1), last.get(j, -1)) + 1
        last[i] = last[j] = lv
        while len(layers) <= lv:
            layers.append([])
        layers[lv].append((ci, i, j))
    return layers


SORT_LAYERS = _layered(SORT_NET)

# Pool TT ops are ~4x slower than DVE 2x-mode ops and consumers stall on
# them, so the sort stays entirely on DVE; Pool computes sum(y) instead.
POOL_OFF = frozenset()


def _rotated_bank(kernel2d):
    """Replicates the reference affine_grid + grid_sample rotation in numpy."""
    lin = np.linspace(-1.0, 1.0, KSIZE)
    xs, ys = np.meshgrid(lin, lin)
    thetas = np.arange(CHANNELS) * np.pi / CHANNELS
    c = np.cos(thetas)[:, None, None]
    s = np.sin(thetas)[:, None, None]
    gx = (c * xs - s * ys).astype(np.float32)
    gy = (s * xs + c * ys).astype(np.float32)

    ix = (gx + np.float32(1.0)) * np.float32(0.5) * np.float32(KSIZE - 1)
    iy = (gy + np.float32(1.0)) * np.float32(0.5) * np.float32(KSIZE - 1)
    ix0 = np.floor(ix)
    iy0 = np.floor(iy)
    ix1 = ix0 + np.float32(1.0)
    iy1 = iy0 + np.float32(1.0)
    wx1 = ix - ix0
    wx0 = np.float32(1.0) - wx1
    wy1 = iy - iy0
    wy0 = np.float32(1.0) - wy1

    def gather(iyc, ixc):
        valid = ((ixc >= 0) & (ixc <= KSIZE - 1) & (iyc >= 0)
                 & (iyc <= KSIZE - 1)).astype(np.float32)
        iyi = np.clip(iyc, 0, KSIZE - 1).astype(np.int32)
        ixi = np.clip(ixc, 0, KSIZE - 1).astype(np.int32)
        return kernel2d[iyi, ixi] * valid

    rot = (gather(iy0, ix0) * wy0 * wx0 + gather(iy0, ix1) * wy0 * wx1 +
           gather(iy1, ix0) * wy1 * wx0 + gather(iy1, ix1) * wy1 * wx1)
    return rot.astype(np.float32)  # (12, 7, 7)


_RUNNER_CACHE = {}


def _build_runner():
    import bass_rust
    import concourse.tile as tile
    from concourse import bacc, mybir

    F32 = mybir.dt.float32
    F16 = mybir.dt.float16
    BF16 = mybir.dt.bfloat16
    Act = mybir.ActivationFunctionType
    Alu = mybir.AluOpType

    def V(pairs):
        return bass_rust.VecI64Pair(pairs)

    nc = bacc.Bacc("TRN2", target_bir_lowering=False, debug=False,
                   enable_asserts=False, num_devices=N_CORES)

    bx_d = nc.dram_tensor("bx", [IMGS, PADW, PADW], BF16, kind="ExternalInput")
    w0_d = nc.dram_tensor("w0", [KDIM, MDIM], BF16, kind="ExternalInput")
    idh_d = nc.dram_tensor("idh", [TB * R, TB * R], F16, kind="ExternalInput")
    # output in tile order: [img, ch, half, p=(r*16+t), f=(sbh*512+w)]
    y_d = nc.dram_tensor("y", [IMGS, CHANNELS + 1, HALVES, TB * R, FDW],
                         F16, kind="ExternalOutput")

    with tile.TileContext(nc) as tc:
        with tc.tile_pool(name="const", bufs=1) as cpool, \
             tc.tile_pool(name="rhs", bufs=4) as rpool, \
             tc.tile_pool(name="stage", bufs=2) as stpool, \
             tc.tile_pool(name="sq", bufs=1) as sqpool, \
             tc.tile_pool(name="ch", bufs=1) as chpool, \
             tc.tile_pool(name="aux", bufs=1) as xpool, \
             tc.tile_pool(name="pc", bufs=2, space="PSUM") as pcpool, \
             tc.tile_pool(name="sums", bufs=1, space="PSUM") as smpool:

            w0 = cpool.tile([KDIM, MDIM], BF16, tag="w0")
            nc.sync.dma_start(w0[:], w0_d.ap())
            idh = cpool.tile([TB * R, TB * R], F16, tag="idh")
            nc.sync.dma_start(idh[:], idh_d.ap())

            for img in range(IMGS):
                for half in range(HALVES):
                    # bf16 staging, block (t, sbh) at column (t*2+sbh)*512
                    stag = stpool.tile([MDIM, SBH * TB * W], F16, tag="stag")
                    stdt = xpool.tile([TB * R, FDW], F16, tag="std", bufs=2)

                    for t in range(TB):
                        pc = pcpool.tile([MDIM, SBH * W], F32, tag="pc")
                        for sbh in range(SBH):
                            sb = half * SBH + sbh
                            # conv rhs straight from HBM (im2col x7 taps):
                            # rhs[q*7+dx, w] = bx[img, sb*128+t*8+q, dx+w]
                            rhs = rpool.tile([KDIM, W], BF16,
                                             tag=f"rhs{sbh}", name="rhs")
                            src = bx_d.ap().copy()
                            src.offset = (img * PADW
                                          + (sb * TB + t) * R) * PADW
                            src.ap = V([[PADW, QROWS], [1, KSIZE], [1, W]])
                            nc.sync.dma_start(rhs[:], src)
                            nc.tensor.matmul(pc[:, sbh * W:(sbh + 1) * W],
                                             w0[:], rhs[:],
                                             start=True, stop=True)
                        # leaky-relu eviction of both blocks at once
                        nc.scalar.activation(
                            stag[:, t * SBH * W:(t + 1) * SBH * W], pc[:],
                            Act.Prelu, alpha=0.01)

                    # gather channel-planar bf16 tiles, one DMA per channel:
                    # ch[p=r*16+t, sbh*512+w] = stag[c*8+r, (t*2+sbh)*512+w]
                    chs = []
                    cur = [0] * CHANNELS

                    def fresh(c):
                        cur[c] ^= 1
                        return chpool.tile([TB * R, FDW], F16,
                                           name=f"srt{c}",
                                           tag=f"{'AB'[cur[c]]}{c}")

                    for c in range(CHANNELS):
                        cht = fresh(c)
                        gsrc = stag[:, :].copy()
                        gsrc.offset = (c * R) * (SBH * TB * W)
                        gsrc.ap = V([[SBH * TB * W, R], [W, SBH * TB],
                                     [1, W]])
                        eng = nc.sync if c % 2 == 0 else nc.scalar
                        eng.dma_start(cht[:], gsrc)
                        chs.append(cht)

                    # sorting network, layer by layer: within a layer all
                    # comparators are independent, so adjacent DVE ops never
                    # stall on each other.  Offloaded comparators compute
                    # max = (a+b) - min with the add/sub on Pool.
                    for layer in SORT_LAYERS:
                        news = {}
                        stiles = {}
                        for ci, i, j in layer:
                            if ci in POOL_OFF:
                                s = xpool.tile([TB * R, FDW], F16,
                                               tag=f"s{ci % 2}", name="s")
                                nc.gpsimd.tensor_tensor(s[:], chs[i][:],
                                                        chs[j][:], Alu.add)
                                stiles[ci] = s
                        for ci, i, j in layer:
                            mnt = fresh(i)
                            nc.vector.tensor_tensor(mnt[:], chs[i][:],
                                                    chs[j][:], Alu.min)
                            news[ci] = mnt
                        for ci, i, j in layer:
                            mxt = fresh(j)
                            if ci in POOL_OFF:
                                nc.gpsimd.tensor_tensor(
                                    mxt[:], stiles[ci][:], news[ci][:],
                                    Alu.subtract)
                            else:
                                nc.vector.tensor_tensor(mxt[:], chs[i][:],
                                                        chs[j][:], Alu.max)
                            chs[i] = news[ci]
                            chs[j] = mxt

                    # tile-order output DMAs (host unpermutes)
                    def out_dma(k, srctile):
                        od = y_d.ap().copy()
                        od.offset = ((img * (CHANNELS + 1) + k) * HALVES
                                     + half) * TB * R * FDW
                        od.ap = V([[FDW, TB * R], [1, FDW]])
                        nc.scalar.dma_start(od, srctile[:])

                    for k in range(CHANNELS):
                        out_dma(k, chs[k])

                    # sums from the SORTED tiles (same multiset as the
                    # inputs): sum(y) as an fp16 add tree on DVE, sum(y^2)
                    # via fp16 squares (ACT) + identity matmuls (PE).
                    acc = list(chs)
                    width = CHANNELS
                    lvl = 0
                    while width > 1:
                        nxt = []
                        for p in range(width // 2):
                            st = sqpool.tile([TB * R, FDW], F16,
                                             tag=f"ps{lvl}_{p}", name="ps")
                            nc.vector.tensor_tensor(st[:], acc[2 * p][:],
                                                    acc[2 * p + 1][:],
                                                    Alu.add)
                            nxt.append(st)
                        if width % 2:
                            nxt.append(acc[-1])
                        acc = nxt
                        width = len(acc)
                        lvl += 1
                    S_t = acc[0]

                    Y_ps = [smpool.tile([TB * R, W], F32, tag=f"Y{i}",
                                        name=f"Y{i}") for i in range(SBH)]
                    sqs = []
                    for c in range(CHANNELS):
                        sq = sqpool.tile([TB * R, FDW], F16, tag=f"sq{c % 3}",
                                         name="sq")
                        nc.scalar.activation(sq[:], chs[c][:], Act.Square)
                        sqs.append(sq)
                    for c in range(CHANNELS):
                        for i in range(SBH):
                            nc.tensor.matmul(Y_ps[i][:], idh[:],
                                             sqs[c][:, i * W:(i + 1) * W],
                                             start=(c == 0),
                                             stop=(c == CHANNELS - 1))

                    # std = sqrt(relu(sum_y2 - S^2/12) / 11)
                    for i in range(SBH):
                        t1 = xpool.tile([TB * R, W], F32, tag="t1")
                        nc.scalar.activation(t1[:],
                                             S_t[:, i * W:(i + 1) * W],
                                             Act.Square)
                        v = xpool.tile([TB * R, W], F32, tag="v")
                        nc.vector.scalar_tensor_tensor(
                            v[:], t1[:], -1.0 / 12.0, Y_ps[i][:],
                            Alu.mult, Alu.add)
                        vc = xpool.tile([TB * R, W], F32, tag="vc")
                        nc.scalar.activation(vc[:], v[:], Act.Relu)
                        nc.scalar.activation(stdt[:, i * W:(i + 1) * W],
                                             vc[:], Act.Sqrt,
                                             scale=1.0 / 11.0)

                    out_dma(CHANNELS, stdt)

    nc.compile()
    return nc


def _get_nc():
    if "nc" not in _RUNNER_CACHE:
        _RUNNER_CACHE["nc"] = _build_runner()
    return _RUNNER_CACHE["nc"]


def _prep_inputs(x, kernel):
    """Host-side prep: rotate bank, build weights, pad + bf16 x."""
    rot = _rotated_bank(np.asarray(kernel, np.float32)[0, 0])

    # lhsT [98, 96]: W[q*7+dx, c*8+r] = rot[c, q-r, dx] for 0 <= q-r <= 6
    Wm = np.zeros((KDIM, MDIM), np.float32)
    for c in range(CHANNELS):
        for r in range(R):
            for dy in range(KSIZE):
                q = r + dy
                for dx in range(KSIZE):
                    Wm[q * KSIZE + dx, c * R + r] = rot[c, dy, dx]
    w0 = Wm.astype(ml_dtypes.bfloat16)
    idh = np.eye(TB * R, dtype=np.float16)

    x = np.asarray(x, np.float32)
    xp = np.zeros((B, PADW, PADW), np.float32)
    xp[:, SIGMA:SIGMA + H, SIGMA:SIGMA + W] = x[:, 0]
    xb = xp.astype(ml_dtypes.bfloat16)

    in_maps = []
    for core in range(N_CORES):
        i0 = core * IMGS
        in_maps.append({
            "bx": xb[i0:i0 + IMGS],
            "w0": w0,
            "idh": idh,
        })
    return in_maps


def run(in_maps, trace=False, **kwargs):
    from concourse import bass_utils
    nc = _get_nc()
    res = bass_utils.run_bass_kernel_spmd(
        nc, in_maps, core_ids=list(range(N_CORES)), trace=trace, **kwargs)
    return res


def _unpermute(y):
    # y: (IMGS, 13, HALVES, 128, 1024) with p = r*16 + t, f = sbh*512 + w
    y = y.reshape(IMGS, CHANNELS + 1, HALVES, R, TB, SBH, W)
    #                    img ch half  r  t  sbh  w -> img ch half sbh t r w
    y = y.transpose(0, 1, 2, 5, 4, 3, 6)
    return y.reshape(IMGS, CHANNELS + 1, H, W)


def kernel(x, kernel):
    in_maps = _prep_inputs(x, kernel)
    res = run(in_maps)
    y = np.stack([_unpermute(np.asarray(res.results[c]["y"]))
                  for c in range(N_CORES)])
    return y.reshape(B, CHANNELS + 1, H, W).astype(np.float32)
